# revision 31
# baseline (speedup 1.0000x reference)
"""BitLinear forward on 8 TRN2 NeuronCores — data-parallel over tokens.

Math: reference computes
    gamma_w = mean|W| + eps;  bw = clip(round(W/gamma_w), -1, 1)
    xn = LayerNorm(x);  gamma = max|xn|;  xq = clip(xn*QB/gamma, +-(QB-eps))
    y  = (xq @ bw.T) * (gamma*beta/QB),  beta = max_d sum_o |W[o,d]|
The gamma factor cancels algebraically, so on device
    y[t,o] = rstd[t]*beta * sum_d (x[d,t]-mu[t]) * bw[d,o]
with NO cross-core collective (collectives downclock the PE 2.4->2.0GHz).

v6 (final: 388us baseline -> ~318us). PE floor = 218us bf16 GEMM +
27us LN-stats matmuls; exec ~= GEMM start + total PE work + feed
stalls, attacked with measured engine rates
(2-input DVE ops ~2.3us per [128,2048] regardless of dtype; 16->16-bit
tensor_scalar 0.69us; ->8-bit out 1.23us; scalar ACT ~0.75us/[128,512];
gpsimd TT ~1.2us/[128,512]; DMA ~270GB/s/queue with ~8us spin-up).

1. W ships twice (layout/dtype prep only):
   - int16 fixed-point (v = round(W/SCALE_W)) in 8 CONTIGUOUS pieces
     (strided pieces cost ~7us extra DMA spin-up) — the precision
     source for the ternary decision (24 flips out of 4.2M, +1e-3 rel).
   - |W| as fp8e4m3, STOCHASTICALLY rounded (RNE's ~0.08% mean bias
     shifts thr -> 823 flips / 1.5e-2 rel; SR is unbiased: 5e-6 gamma
     err), TRANSPOSED to [o, d] tiles, 4.2MB.  Sum_o|W| then comes
     from ones-matmuls on the otherwise-idle PE during the load
     window (broadcast over partitions), so gamma-total AND beta are
     two cheap free-axis psum reduces on Vector.  No abs chase on
     Vector/Scalar, no partition allreduce, nothing on GpSimd.
2. Ternarize = 2 fast single-input ops (any 2-input DVE op is 3x
   slower): q16 = round(v * 1/two_thr) via int16-convert rounding
   (round-half-even, matches jnp.round); bw = clip(q16,-1,1) -> fp8e4.
   1.9us/tile in k-order [8..15, 0..7], chasing the int16 stream that
   lands split across both DMA queues; the GEMM consumes in the same
   k-order.
3. Queues: sync = [w8 x16 | w16 k0-7 | xc1 | xc2 | xc3 | y-outs];
   scalar HWDGE = [xc0 | w16 k8-15 | rstd columnize round-trips].
   xc1 rides sync BEHIND the w16 stream: keeping it out of the scalar
   queue's critical window lets w8 (which gates thr) land earlier.
   (Splitting w8 itself across both queues was tried twice and LOSES:
   it starves the w16/x stream and the tern feed pays more than thr
   gains.)
   rstd is columnized by a tiny SBUF->DRAM->SBUF round-trip (f32
   exact) instead of PE transposes: that keeps PSUM free for the 8
   GEMM banks (a PE-transpose here deadlock-prone: its bank alloc
   waits on banks the epilogue frees, and the epilogue waits on rbb).
4. Engine roles: PE = wsum mms (pre-GEMM window) + stats mms + GEMM;
   Vector = sq(c0-half,c2,c3), thr reduces, ternarize, var/rstd, rbb;
   Scalar = sq(c0-half,c1), mu broadcasts, sqrt, PSUM-drain epilogue;
   GpSimd = mean-subtracts only (k-ordered, chases ahead of the GEMM).
   Chunk m's sq/stats/mu/var/subs and sd/rstd/rbb ride as two separate
   inserts at hand-picked emission points inside earlier chunks' GEMM
   so no in-order FIFO ever blocks another engine's critical chain.
5. First two m-tiles of chunk 0 run k-interleaved across all 8 PSUM
   banks, pacing the PE to the ternarize trickle (DVE ops run ~2x
   slower while DMA loads hit SBUF, so the trickle is real); chunk-0
   stats matmuls are emission-interleaved into the wsum stream's DMA
   gaps.  The last m-tile runs its k-loop c-major so its four banks
   finish and drain progressively (muls split Vector/Scalar, DMAs
   split across both queues), cutting the tail to ~6us.
6. w8 tiles are fully resident (bufs=KT): with a smaller ring, each
   piece's DMA trigger waits on PE-consumption sems and the in-order
   sync queue degenerates into a ~2.5us/piece round-trip lockstep
   that delays thr (and everything behind it) by ~25us.
"""

import os
import sys

import numpy as np

for _p in ("/opt/trn_rl_repo", "/root/.axon_site/_ro/trn_rl_repo"):
    if os.path.isdir(_p) and _p not in sys.path:
        sys.path.append(_p)

from concourse import bacc, mybir, tile  # noqa: E402
from concourse.bass_utils import run_bass_kernel_spmd  # noqa: E402

P = 128
D = 2048  # contraction (hidden) dim
O = 2048  # output dim
N_CORES = 8
N_TOK = 4 * 4096
TOK = N_TOK // N_CORES  # 2048 tokens per core
KT = D // P  # 16 contraction tiles
CW = 512  # token-chunk width
NC_CHUNK = TOK // CW  # 4 chunks
MT = TOK // P  # 16 m-tiles per core
CH = 512  # psum free chunk (one bank of f32)
NCH = O // CH
NWQ = 8  # int16 W arrives in 8 pieces of 2 k-tiles
EPS = 1e-5
BOUND = 1.0 / np.sqrt(D)
SCALE_W = BOUND / 32767.0
F32 = mybir.dt.float32
BF16 = mybir.dt.bfloat16
FP8 = mybir.dt.float8e4
I16 = mybir.dt.int16

KS_ORDER = list(range(8, 16)) + list(range(8))  # k8-15 land first


def build_nc():
    nc = bacc.Bacc(None, target_bir_lowering=False, debug=False)
    xc = nc.declare_dram_parameter("xc", [NC_CHUNK * P, KT * CW], BF16, isOutput=False)
    wi = nc.declare_dram_parameter("wi", [NWQ * P, 2 * O], I16, isOutput=False)
    w8 = nc.declare_dram_parameter("w8", [KT * P, D], FP8, isOutput=False)
    y = nc.declare_dram_parameter("y", [TOK, O], F32, isOutput=True)

    Alu = mybir.AluOpType
    Act = mybir.ActivationFunctionType
    Ax = mybir.AxisListType

    with tile.TileContext(nc) as tc:
        with (
            tc.tile_pool(name="const", bufs=1) as const,
            tc.tile_pool(name="xb01", bufs=2) as xb01,
            tc.tile_pool(name="sq", bufs=6) as sqp,
            tc.tile_pool(name="bw", bufs=KT) as bwp,
            tc.tile_pool(name="q16", bufs=2) as q16p,
            tc.tile_pool(name="mub", bufs=2) as mubp,
            tc.tile_pool(name="fin", bufs=4) as fpool,
            tc.tile_pool(name="ypool", bufs=3) as ypool,
            tc.tile_pool(name="dram", bufs=4, space="DRAM") as dpool,
            tc.tile_pool(name="psum", bufs=8, space="PSUM") as psum,
        ):
            xb = [None] * NC_CHUNK  # [P, KT*CW] bf16; slice k via [:, k*CW:]

            ones_b = const.tile([P, P], BF16)
            eps_t = const.tile([P, 1], F32)
            scal = const.tile([P, 8], F32)  # scalar registry (columns)
            wred = const.tile([P, 8], F32)  # psum-reduce partials
            rbinv = const.tile([P, MT], F32)  # rstd columnized
            rbb = const.tile([P, MT], F32)  # rstd * beta columnized

            # ---------------- emission helpers ----------------
            def sq_tiles(m, ks, eng):
                out = {}
                for k in ks:
                    sq = sqp.tile([P, CW], BF16, tag="sq", name=f"sq{m}_{k}")
                    xs = xb[m][:, CW * k : CW * (k + 1)]
                    if eng == "v":
                        nc.vector.tensor_tensor(out=sq, in0=xs, in1=xs, op=Alu.mult)
                    else:
                        nc.scalar.activation(sq, xs, Act.Square)
                    out[k] = sq
                return out

            def stats_mms(m, sqs):
                ps_mu = psum.tile([P, CW], F32, tag="ps", name=f"ps_mu{m}")
                ps_sq = psum.tile([P, CW], F32, tag="ps", name=f"ps_sq{m}")
                for k in range(KT):
                    first, last = k == 0, k == KT - 1
                    nc.tensor.matmul(
                        ps_mu, ones_b, xb[m][:, CW * k : CW * (k + 1)],
                        start=first, stop=last,
                    )
                    nc.tensor.matmul(ps_sq, ones_b, sqs[k], start=first, stop=last)
                return ps_mu, ps_sq

            def mu_var_part(m, ps_mu, ps_sq):
                """Scalar: mu broadcast + mu^2; Vector: var (frees psum fast)."""
                mu_b = mubp.tile([P, CW], BF16, tag="mub")
                nc.scalar.mul(mu_b, ps_mu, 1.0 / D)
                musq = fpool.tile([P, CW], F32, tag="fin", name=f"musq{m}")
                nc.scalar.activation(musq, mu_b, Act.Square)
                var_f = fpool.tile([P, CW], F32, tag="fin", name=f"var{m}")
                nc.vector.scalar_tensor_tensor(
                    out=var_f, in0=ps_sq, scalar=1.0 / D, in1=musq,
                    op0=Alu.mult, op1=Alu.subtract,
                )
                return mu_b, var_f

            def subs_part(m, mu_b):
                # mean-subtract in place on GpSimd, GEMM k-order
                for k in KS_ORDER:
                    xs = xb[m][:, CW * k : CW * (k + 1)]
                    nc.gpsimd.tensor_tensor(out=xs, in0=xs, in1=mu_b, op=Alu.subtract)

            def rstd_part(m, var_f):
                sd_f = fpool.tile([P, CW], F32, tag="fin", name=f"sd{m}")
                nc.scalar.activation(sd_f, var_f, Act.Sqrt, bias=eps_t)
                rstd_f = fpool.tile([P, CW], F32, tag="fin", name=f"rstd{m}")
                nc.vector.reciprocal(rstd_f, sd_f)
                # columnize via tiny DRAM round-trip on the scalar queue
                dcol = dpool.tile([4, P], F32, name=f"dcol{m}")
                nc.scalar.dma_start(dcol, rstd_f[0:1, :])
                nc.scalar.dma_start(
                    rbinv[:, 4 * m : 4 * (m + 1)], dcol.rearrange("a b -> b a")
                )
                return rstd_f

            def rbb_part(m):
                # rbb col AFTER beta write (program order = dep direction)
                nc.vector.tensor_scalar(
                    out=rbb[:, 4 * m : 4 * (m + 1)],
                    in0=rbinv[:, 4 * m : 4 * (m + 1)],
                    scalar1=scal[:, 3:4], scalar2=None, op0=Alu.mult,
                )

            # ================= prologue =================
            with tc.tile_pool(name="w16", bufs=6) as w16p:
                # scalar HWDGE queue: xc0, w16 k8-15, xc1
                xb[0] = xb01.tile([P, KT * CW], BF16, name="xc0")
                nc.scalar.dma_start(xb[0], xc[0:P, :])
                w16t = [None] * NWQ  # piece q holds k=2q, 2q+1
                for q in range(4, NWQ):
                    wt = w16p.tile([P, 2 * O], I16, tag="w", name=f"w{q}")
                    nc.scalar.dma_start(wt, wi[P * q : P * (q + 1), :])
                    w16t[q] = wt
                nc.vector.memset(ones_b, 1.0)
                nc.vector.memset(eps_t, EPS)

                with tc.tile_pool(name="w8", bufs=KT) as w8p:
                    # sync queue: w8 x16, then w16 k0-7
                    w8t = []
                    for t in range(KT):
                        w = w8p.tile([P, D], FP8, tag="w8", name=f"w8_{t}")
                        nc.sync.dma_start(w, w8[P * t : P * (t + 1), :])
                        w8t.append(w)
                    for q in range(4):
                        wt = w16p.tile([P, 2 * O], I16, tag="w", name=f"w{q}")
                        nc.sync.dma_start(wt, wi[P * q : P * (q + 1), :])
                        w16t[q] = wt
                    # xc1 rides sync BEHIND the w16 stream: keeping it off
                    # the scalar queue's critical window lets w8 (which
                    # gates thr -> ternarize -> everything) land ~16us
                    # earlier; chunk-1 stats only need xc1 at ~55us
                    xb[1] = xb01.tile([P, KT * CW], BF16, name="xc1")
                    nc.sync.dma_start(xb[1], xc[P : 2 * P, :])

                    # Sum_o |W| via PE: 4 psum banks, chasing the w8
                    # stream, with chunk-0 stats matmuls slotted into the
                    # stream's tail where the PE would otherwise wait
                    ps_w = [
                        psum.tile([P, CH], F32, tag="ps", name=f"ps_w{c}")
                        for c in range(4)
                    ]

                    def wsum_mms(ts):
                        for t in ts:
                            for c in range(4):
                                nc.tensor.matmul(
                                    ps_w[c], ones_b,
                                    w8t[t][:, CH * c : CH * (c + 1)],
                                    start=(t == 0), stop=(t == KT - 1),
                                )

                    sq0 = sq_tiles(0, range(8), "v")
                    sq0.update(sq_tiles(0, range(8, KT), "s"))
                    wsum_mms(range(12))
                    ps_mu0 = psum.tile([P, CW], F32, tag="ps", name="ps_mu0")
                    ps_sq0 = psum.tile([P, CW], F32, tag="ps", name="ps_sq0")
                    for k in range(8):
                        nc.tensor.matmul(
                            ps_mu0, ones_b, xb[0][:, CW * k : CW * (k + 1)],
                            start=(k == 0), stop=False,
                        )
                        nc.tensor.matmul(
                            ps_sq0, ones_b, sq0[k], start=(k == 0), stop=False
                        )
                    wsum_mms(range(12, KT))
                    for k in range(8, KT):
                        nc.tensor.matmul(
                            ps_mu0, ones_b, xb[0][:, CW * k : CW * (k + 1)],
                            start=False, stop=(k == KT - 1),
                        )
                        nc.tensor.matmul(
                            ps_sq0, ones_b, sq0[k], start=False, stop=(k == KT - 1)
                        )
                pm0, psq0 = ps_mu0, ps_sq0

                # thr path: free-axis reduces of the broadcast psum
                for c in range(4):
                    nc.vector.tensor_reduce(
                        wred[:, c : c + 1], ps_w[c], axis=Ax.X, op=Alu.add
                    )
                for c in range(4):
                    nc.vector.tensor_reduce(
                        wred[:, 4 + c : 5 + c], ps_w[c], axis=Ax.X, op=Alu.max
                    )
                tot_b = scal[:, 2:3]
                nc.vector.tensor_reduce(tot_b, wred[:, 0:4], axis=Ax.X, op=Alu.add)
                beta_b = scal[:, 3:4]
                nc.vector.tensor_reduce(beta_b, wred[:, 4:8], axis=Ax.X, op=Alu.max)
                # two_thr = gamma_w/SCALE_W = tot/(D*O*SCALE_W) + EPS/SCALE_W
                two_thr = scal[:, 4:5]
                nc.scalar.activation(
                    two_thr, tot_b, Act.Copy,
                    bias=float(EPS / SCALE_W), scale=float(1.0 / (D * O * SCALE_W)),
                )
                inv2t = scal[:, 5:6]
                nc.vector.reciprocal(inv2t, two_thr)

                # chunk-0 mu / subtracts (gpsimd chases ahead of the GEMM)
                mu0, var0 = mu_var_part(0, pm0, psq0)
                subs_part(0, mu0)

                # ---- ternarize: q16 = round(v*inv2t); bw = clip -> fp8
                bwt = [None] * KT

                def tern(k):
                    wk = w16t[k // 2][:, O * (k % 2) : O * (k % 2 + 1)]
                    q16 = q16p.tile([P, O], I16, tag="q16")
                    nc.vector.tensor_scalar(
                        out=q16, in0=wk, scalar1=inv2t, scalar2=None, op0=Alu.mult
                    )
                    bwk = bwp.tile([P, O], FP8, tag="bw", name=f"bw{k}")
                    nc.vector.tensor_scalar(
                        out=bwk, in0=q16, scalar1=-1.0, scalar2=1.0,
                        op0=Alu.max, op1=Alu.min,
                    )
                    bwt[k] = bwk

                for k in KS_ORDER[:4]:
                    tern(k)
                rstd_part(0, var0)  # var already computed; sd/rstd/columnize
                for k in KS_ORDER[4:]:
                    tern(k)
                rbb_part(0)  # executes once beta + columnize land

            # ---- w16 released: its SBUF hosts chunks 2,3 -------------
            with tc.tile_pool(name="xb23", bufs=2) as xb23:
                xb[2] = xb23.tile([P, KT * CW], BF16, name="xc2")
                nc.sync.dma_start(xb[2], xc[2 * P : 3 * P, :])
                xb[3] = xb23.tile([P, KT * CW], BF16, name="xc3")
                nc.sync.dma_start(xb[3], xc[3 * P : 4 * P, :])

                def epilogue(m, j, pys):
                    g = 4 * m + j
                    for c in range(NCH):
                        ysb = ypool.tile([P, CH], F32, tag="y")
                        nc.scalar.mul(ysb, pys[c], rbb[:, g : g + 1])
                        nc.sync.dma_start(
                            y[P * g : P * (g + 1), CH * c : CH * (c + 1)], ysb
                        )

                def gemm_pair(m, j0, j1):
                    """Two m-tiles k-interleaved across all 8 PSUM banks,
                    pacing the PE to the ternarize trickle."""
                    js = [j0, j1]
                    pys = {}
                    for j in js:
                        g = 4 * m + j
                        pys[j] = [
                            psum.tile([P, CH], F32, tag="ps", name=f"py{g}_{c}")
                            for c in range(NCH)
                        ]
                    for i, k in enumerate(KS_ORDER):
                        first, last = i == 0, i == KT - 1
                        for j in js:
                            lhs = xb[m][:, CW * k + P * j : CW * k + P * (j + 1)]
                            for c in range(NCH):
                                nc.tensor.matmul(
                                    pys[j][c], lhs,
                                    bwt[k][:, CH * c : CH * (c + 1)],
                                    start=first, stop=last,
                                )
                    for j in js:
                        epilogue(m, j, pys[j])

                def gemm_one(m, j, pre=None, last=False):
                    if pre:
                        pre()
                    g = 4 * m + j
                    pys = [
                        psum.tile([P, CH], F32, tag="ps", name=f"py{g}_{c}")
                        for c in range(NCH)
                    ]
                    if not last:
                        for i, k in enumerate(KS_ORDER):
                            lhs = xb[m][:, CW * k + P * j : CW * k + P * (j + 1)]
                            first, last_k = i == 0, i == KT - 1
                            for c in range(NCH):
                                nc.tensor.matmul(
                                    pys[c], lhs, bwt[k][:, CH * c : CH * (c + 1)],
                                    start=first, stop=last_k,
                                )
                        epilogue(m, j, pys)
                        return
                    # final m-tile: c-major so banks finish (and drain)
                    # progressively; split muls/DMAs across engines/queues
                    engs = (
                        (nc.vector, nc.sync), (nc.scalar, nc.scalar),
                        (nc.vector, nc.sync), (nc.scalar, nc.scalar),
                    )
                    for c in range(NCH):
                        for i, k in enumerate(KS_ORDER):
                            lhs = xb[m][:, CW * k + P * j : CW * k + P * (j + 1)]
                            nc.tensor.matmul(
                                pys[c], lhs, bwt[k][:, CH * c : CH * (c + 1)],
                                start=(i == 0), stop=(i == KT - 1),
                            )
                        ysb = ypool.tile([P, CH], F32, tag="y", name=f"yfin{c}")
                        meng, deng = engs[c]
                        if meng is nc.vector:
                            nc.vector.tensor_scalar(
                                out=ysb, in0=pys[c], scalar1=rbb[:, g : g + 1],
                                scalar2=None, op0=Alu.mult,
                            )
                        else:
                            nc.scalar.mul(ysb, pys[c], rbb[:, g : g + 1])
                        deng.dma_start(
                            y[P * g : P * (g + 1), CH * c : CH * (c + 1)], ysb
                        )

                carry = {}

                def stats_ins(m, sq_eng):
                    def _f():
                        sqs = sq_tiles(m, range(KT), sq_eng)
                        pm, psq = stats_mms(m, sqs)
                        mu_m, var_m = mu_var_part(m, pm, psq)
                        subs_part(m, mu_m)
                        carry[m] = var_m
                    return _f

                def fin_ins(m):
                    def _f():
                        rstd_part(m, carry.pop(m))
                        rbb_part(m)
                    return _f

                gemm_pair(0, 0, 1)
                gemm_one(0, 2, pre=stats_ins(1, "s"))
                gemm_one(0, 3, pre=fin_ins(1))
                gemm_one(1, 0, pre=stats_ins(2, "v"))
                gemm_one(1, 1, pre=fin_ins(2))
                gemm_one(1, 2, pre=stats_ins(3, "v"))
                gemm_one(1, 3, pre=fin_ins(3))
                for j in range(4):
                    gemm_one(2, j)
                for j in range(4):
                    gemm_one(3, j, last=(j == 3))

    nc.compile()
    return nc


_NC_CACHE = None


def _get_nc():
    global _NC_CACHE
    if _NC_CACHE is None:
        _NC_CACHE = build_nc()
    return _NC_CACHE


def _sr_fp8_abs(a, seed=12345):
    """Stochastic-round |a| to fp8e4m3 (unbiased, unlike RNE)."""
    import ml_dtypes

    aw = np.abs(a).astype(np.float32)
    f8 = aw.astype(ml_dtypes.float8_e4m3fn)
    lo_c = f8.astype(np.float32)
    hi = np.nextafter(f8, ml_dtypes.float8_e4m3fn(448)).astype(np.float32)
    lo_dn = np.nextafter(f8, ml_dtypes.float8_e4m3fn(0)).astype(np.float32)
    lo3 = np.where(lo_c <= aw, lo_c, lo_dn)
    hi2 = np.where(lo_c >= aw, lo_c, hi)
    span = np.where(hi2 > lo3, hi2 - lo3, 1.0)
    p = (aw - lo3) / span
    rng = np.random.default_rng(seed)
    out = np.where(rng.random(aw.shape) < p, hi2, lo3)
    return out.astype(ml_dtypes.float8_e4m3fn)


def _prep_in_maps(x, fweight):
    import ml_dtypes

    x2 = np.asarray(x, dtype=np.float32).reshape(N_TOK, D).astype(ml_dtypes.bfloat16)
    fw = np.asarray(fweight, dtype=np.float32)
    fwt = fw.T  # [D, O]
    wi = np.round(fwt / SCALE_W).clip(-32767, 32767).astype(np.int16)
    # contiguous pieces: piece q rows = k-tiles 2q,2q+1 -> [8*128, 4096]
    wi = np.ascontiguousarray(
        wi.reshape(NWQ, 2, P, O).transpose(0, 2, 1, 3).reshape(NWQ * P, 2 * O)
    )
    w8 = np.ascontiguousarray(_sr_fp8_abs(fw))  # [O, D] = [16*128, 2048]
    in_maps = []
    for c in range(N_CORES):
        xs = x2[c * TOK : (c + 1) * TOK, :]  # [TOK, D] bf16
        chunks = []
        for m in range(NC_CHUNK):
            blk = xs[m * CW : (m + 1) * CW, :].T  # [D, CW]
            chunks.append(
                blk.reshape(KT, P, CW).transpose(1, 0, 2).reshape(P, KT * CW)
            )
        xch = np.ascontiguousarray(np.concatenate(chunks, axis=0))
        in_maps.append({"xc": xch, "wi": wi, "w8": w8})
    return in_maps


def run_spmd(x, fweight, **kw):
    nc = _get_nc()
    in_maps = _prep_in_maps(x, fweight)
    return run_bass_kernel_spmd(nc, in_maps, core_ids=list(range(N_CORES)), **kw)


def kernel(x, fweight):
    res = run_spmd(x, fweight)
    y = np.concatenate([res.results[c]["y"] for c in range(N_CORES)], axis=0)
    return y.reshape(4, 4096, O)


if __name__ == "__main__":
    xx = np.random.randn(4, 4096, D).astype(np.float32)
    ww = np.random.uniform(-1 / np.sqrt(D), 1 / np.sqrt(D), (O, D)).astype(np.float32)
    out = kernel(xx, ww)
    print("out", out.shape, out.dtype, float(np.abs(out).mean()))


# revision 32
# speedup vs baseline: 1.0097x; 1.0097x over previous
"""BitLinear forward on 8 TRN2 NeuronCores — data-parallel over tokens.

Math: reference computes
    gamma_w = mean|W| + eps;  bw = clip(round(W/gamma_w), -1, 1)
    xn = LayerNorm(x);  gamma = max|xn|;  xq = clip(xn*QB/gamma, +-(QB-eps))
    y  = (xq @ bw.T) * (gamma*beta/QB),  beta = max_d sum_o |W[o,d]|
The gamma factor cancels algebraically, so on device
    y[t,o] = rstd[t]*beta * sum_d (x[d,t]-mu[t]) * bw[d,o]
with NO cross-core collective (collectives downclock the PE 2.4->2.0GHz).

v6 (final: 388us baseline -> ~318us). PE floor = 218us bf16 GEMM +
27us LN-stats matmuls; exec ~= GEMM start + total PE work + feed
stalls, attacked with measured engine rates
(2-input DVE ops ~2.3us per [128,2048] regardless of dtype; 16->16-bit
tensor_scalar 0.69us; ->8-bit out 1.23us; scalar ACT ~0.75us/[128,512];
gpsimd TT ~1.2us/[128,512]; DMA ~270GB/s/queue with ~8us spin-up).

1. W ships twice (layout/dtype prep only):
   - int16 fixed-point (v = round(W/SCALE_W)) in 8 CONTIGUOUS pieces
     (strided pieces cost ~7us extra DMA spin-up) — the precision
     source for the ternary decision (24 flips out of 4.2M, +1e-3 rel).
   - |W| as fp8e4m3, STOCHASTICALLY rounded (RNE's ~0.08% mean bias
     shifts thr -> 823 flips / 1.5e-2 rel; SR is unbiased: 5e-6 gamma
     err), TRANSPOSED to [o, d] tiles, 4.2MB.  Sum_o|W| then comes
     from ones-matmuls on the otherwise-idle PE during the load
     window (broadcast over partitions), so gamma-total AND beta are
     two cheap free-axis psum reduces on Vector.  No abs chase on
     Vector/Scalar, no partition allreduce, nothing on GpSimd.
2. Ternarize = 2 fast single-input ops (any 2-input DVE op is 3x
   slower): q16 = round(v * 1/two_thr) via int16-convert rounding
   (round-half-even, matches jnp.round); bw = clip(q16,-1,1) -> fp8e4.
   1.9us/tile in k-order [8..15, 0..7], chasing the int16 stream that
   lands split across both DMA queues; the GEMM consumes in the same
   k-order.
3. Queues: sync = [w8 x16 | w16 k0-7 | xc1 | xc2 | xc3 | y-outs];
   scalar HWDGE = [xc0 | w16 k8-15 | rstd columnize round-trips].
   xc1 rides sync BEHIND the w16 stream: keeping it out of the scalar
   queue's critical window lets w8 (which gates thr) land earlier.
   (Splitting w8 itself across both queues was tried twice and LOSES:
   it starves the w16/x stream and the tern feed pays more than thr
   gains.)
   rstd is columnized by a tiny SBUF->DRAM->SBUF round-trip (f32
   exact) instead of PE transposes: that keeps PSUM free for the 8
   GEMM banks (a PE-transpose here deadlock-prone: its bank alloc
   waits on banks the epilogue frees, and the epilogue waits on rbb).
4. Engine roles: PE = wsum mms (pre-GEMM window) + stats mms + GEMM;
   Vector = sq(c0-half,c2,c3), thr reduces, ternarize, var/rstd, rbb;
   Scalar = sq(c0-half,c1), mu broadcasts, sqrt, PSUM-drain epilogue;
   GpSimd = mean-subtracts only (k-ordered, chases ahead of the GEMM).
   Chunk m's sq/stats/mu/var/subs and sd/rstd/rbb ride as two separate
   inserts at hand-picked emission points inside earlier chunks' GEMM
   so no in-order FIFO ever blocks another engine's critical chain.
5. First two m-tiles of chunk 0 run k-interleaved across all 8 PSUM
   banks, pacing the PE to the ternarize trickle (DVE ops run ~2x
   slower while DMA loads hit SBUF, so the trickle is real); chunk-0
   stats matmuls are emission-interleaved into the wsum stream's DMA
   gaps.  The last m-tile runs its k-loop c-major so its four banks
   finish and drain progressively (muls split Vector/Scalar, DMAs
   split across both queues), cutting the tail to ~6us.
6. w8 tiles are fully resident (bufs=KT): with a smaller ring, each
   piece's DMA trigger waits on PE-consumption sems and the in-order
   sync queue degenerates into a ~2.5us/piece round-trip lockstep
   that delays thr (and everything behind it) by ~25us.
"""

import os
import sys

import numpy as np

for _p in ("/opt/trn_rl_repo", "/root/.axon_site/_ro/trn_rl_repo"):
    if os.path.isdir(_p) and _p not in sys.path:
        sys.path.append(_p)

from concourse import bacc, mybir, tile  # noqa: E402
from concourse.bass_utils import run_bass_kernel_spmd  # noqa: E402

P = 128
D = 2048  # contraction (hidden) dim
O = 2048  # output dim
N_CORES = 8
N_TOK = 4 * 4096
TOK = N_TOK // N_CORES  # 2048 tokens per core
KT = D // P  # 16 contraction tiles
CW = 512  # token-chunk width
NC_CHUNK = TOK // CW  # 4 chunks
MT = TOK // P  # 16 m-tiles per core
CH = 512  # psum free chunk (one bank of f32)
NCH = O // CH
NWQ = 8  # int16 W arrives in 8 pieces of 2 k-tiles
EPS = 1e-5
BOUND = 1.0 / np.sqrt(D)
SCALE_W = BOUND / 32767.0
F32 = mybir.dt.float32
BF16 = mybir.dt.bfloat16
FP8 = mybir.dt.float8e4
I16 = mybir.dt.int16

KS_ORDER = list(range(8, 16)) + list(range(8))  # k8-15 land first


def build_nc():
    nc = bacc.Bacc(None, target_bir_lowering=False, debug=False)
    xc = nc.declare_dram_parameter("xc", [NC_CHUNK * P, KT * CW], BF16, isOutput=False)
    wi = nc.declare_dram_parameter("wi", [NWQ * P, 2 * O], I16, isOutput=False)
    w8 = nc.declare_dram_parameter("w8", [KT * P, D], FP8, isOutput=False)
    y = nc.declare_dram_parameter("y", [TOK, O], F32, isOutput=True)

    Alu = mybir.AluOpType
    Act = mybir.ActivationFunctionType
    Ax = mybir.AxisListType

    with tile.TileContext(nc) as tc:
        with (
            tc.tile_pool(name="const", bufs=1) as const,
            tc.tile_pool(name="xb01", bufs=2) as xb01,
            tc.tile_pool(name="sq", bufs=4) as sqp,
            tc.tile_pool(name="bw", bufs=KT) as bwp,
            tc.tile_pool(name="q16", bufs=2) as q16p,
            tc.tile_pool(name="mub", bufs=2) as mubp,
            tc.tile_pool(name="fin", bufs=4) as fpool,
            tc.tile_pool(name="ypool", bufs=3) as ypool,
            tc.tile_pool(name="dram", bufs=4, space="DRAM") as dpool,
            tc.tile_pool(name="psum", bufs=8, space="PSUM") as psum,
        ):
            xb = [None] * NC_CHUNK  # [P, KT*CW] bf16; slice k via [:, k*CW:]

            ones_b = const.tile([P, P], BF16)
            eps_t = const.tile([P, 1], F32)
            scal = const.tile([P, 8], F32)  # scalar registry (columns)
            wred = const.tile([P, 8], F32)  # psum-reduce partials
            rbinv = const.tile([P, MT], F32)  # rstd columnized
            rbb = const.tile([P, MT], F32)  # rstd * beta columnized

            # ---------------- emission helpers ----------------
            def sq_tiles(m, ks, eng):
                out = {}
                for k in ks:
                    sq = sqp.tile([P, CW], BF16, tag="sq", name=f"sq{m}_{k}")
                    xs = xb[m][:, CW * k : CW * (k + 1)]
                    if eng == "v":
                        nc.vector.tensor_tensor(out=sq, in0=xs, in1=xs, op=Alu.mult)
                    else:
                        nc.scalar.activation(sq, xs, Act.Square)
                    out[k] = sq
                return out

            def stats_mms(m, sqs):
                ps_mu = psum.tile([P, CW], F32, tag="ps", name=f"ps_mu{m}")
                ps_sq = psum.tile([P, CW], F32, tag="ps", name=f"ps_sq{m}")
                for k in range(KT):
                    first, last = k == 0, k == KT - 1
                    nc.tensor.matmul(
                        ps_mu, ones_b, xb[m][:, CW * k : CW * (k + 1)],
                        start=first, stop=last,
                    )
                    nc.tensor.matmul(ps_sq, ones_b, sqs[k], start=first, stop=last)
                return ps_mu, ps_sq

            def mu_var_part(m, ps_mu, ps_sq):
                """Scalar: mu broadcast + mu^2; Vector: var (frees psum fast)."""
                mu_b = mubp.tile([P, CW], BF16, tag="mub")
                nc.scalar.mul(mu_b, ps_mu, 1.0 / D)
                musq = fpool.tile([P, CW], F32, tag="fin", name=f"musq{m}")
                nc.scalar.activation(musq, mu_b, Act.Square)
                var_f = fpool.tile([P, CW], F32, tag="fin", name=f"var{m}")
                nc.vector.scalar_tensor_tensor(
                    out=var_f, in0=ps_sq, scalar=1.0 / D, in1=musq,
                    op0=Alu.mult, op1=Alu.subtract,
                )
                return mu_b, var_f

            def subs_part(m, mu_b):
                # mean-subtract in place on GpSimd, GEMM k-order
                for k in KS_ORDER:
                    xs = xb[m][:, CW * k : CW * (k + 1)]
                    nc.gpsimd.tensor_tensor(out=xs, in0=xs, in1=mu_b, op=Alu.subtract)

            def rstd_part(m, var_f):
                sd_f = fpool.tile([P, CW], F32, tag="fin", name=f"sd{m}")
                nc.scalar.activation(sd_f, var_f, Act.Sqrt, bias=eps_t)
                rstd_f = fpool.tile([P, CW], F32, tag="fin", name=f"rstd{m}")
                nc.vector.reciprocal(rstd_f, sd_f)
                # columnize via tiny DRAM round-trip on the scalar queue
                dcol = dpool.tile([4, P], F32, name=f"dcol{m}")
                nc.scalar.dma_start(dcol, rstd_f[0:1, :])
                nc.scalar.dma_start(
                    rbinv[:, 4 * m : 4 * (m + 1)], dcol.rearrange("a b -> b a")
                )
                return rstd_f

            def rbb_part(m):
                # rbb col AFTER beta write (program order = dep direction)
                nc.vector.tensor_scalar(
                    out=rbb[:, 4 * m : 4 * (m + 1)],
                    in0=rbinv[:, 4 * m : 4 * (m + 1)],
                    scalar1=scal[:, 3:4], scalar2=None, op0=Alu.mult,
                )

            # ================= prologue =================
            with tc.tile_pool(name="w16", bufs=6) as w16p:
                # scalar HWDGE queue: xc0, w16 k8-15, xc1
                xb[0] = xb01.tile([P, KT * CW], BF16, name="xc0")
                nc.scalar.dma_start(xb[0], xc[0:P, :])
                w16t = [None] * NWQ  # piece q holds k=2q, 2q+1
                for q in range(4, NWQ):
                    wt = w16p.tile([P, 2 * O], I16, tag="w", name=f"w{q}")
                    nc.scalar.dma_start(wt, wi[P * q : P * (q + 1), :])
                    w16t[q] = wt
                nc.vector.memset(ones_b, 1.0)
                nc.vector.memset(eps_t, EPS)

                with tc.tile_pool(name="w8", bufs=KT) as w8p:
                    # sync queue: w8 x16, then w16 k0-7
                    w8t = []
                    for t in range(KT):
                        w = w8p.tile([P, D], FP8, tag="w8", name=f"w8_{t}")
                        nc.sync.dma_start(w, w8[P * t : P * (t + 1), :])
                        w8t.append(w)
                    for q in range(4):
                        wt = w16p.tile([P, 2 * O], I16, tag="w", name=f"w{q}")
                        nc.sync.dma_start(wt, wi[P * q : P * (q + 1), :])
                        w16t[q] = wt
                    # xc1 rides sync BEHIND the w16 stream: keeping it off
                    # the scalar queue's critical window lets w8 (which
                    # gates thr -> ternarize -> everything) land ~16us
                    # earlier; chunk-1 stats only need xc1 at ~55us
                    xb[1] = xb01.tile([P, KT * CW], BF16, name="xc1")
                    nc.sync.dma_start(xb[1], xc[P : 2 * P, :])

                    # Sum_o |W| via PE: 4 psum banks, chasing the w8
                    # stream, with chunk-0 stats matmuls slotted into the
                    # stream's tail where the PE would otherwise wait
                    ps_w = [
                        psum.tile([P, CH], F32, tag="ps", name=f"ps_w{c}")
                        for c in range(4)
                    ]

                    def wsum_mms(ts):
                        for t in ts:
                            for c in range(4):
                                nc.tensor.matmul(
                                    ps_w[c], ones_b,
                                    w8t[t][:, CH * c : CH * (c + 1)],
                                    start=(t == 0), stop=(t == KT - 1),
                                )

                    sq0 = sq_tiles(0, range(8), "v")
                    sq0.update(sq_tiles(0, range(8, KT), "s"))
                    wsum_mms(range(12))
                    ps_mu0 = psum.tile([P, CW], F32, tag="ps", name="ps_mu0")
                    ps_sq0 = psum.tile([P, CW], F32, tag="ps", name="ps_sq0")
                    for k in range(8):
                        nc.tensor.matmul(
                            ps_mu0, ones_b, xb[0][:, CW * k : CW * (k + 1)],
                            start=(k == 0), stop=False,
                        )
                        nc.tensor.matmul(
                            ps_sq0, ones_b, sq0[k], start=(k == 0), stop=False
                        )
                    wsum_mms(range(12, KT))
                    for k in range(8, KT):
                        nc.tensor.matmul(
                            ps_mu0, ones_b, xb[0][:, CW * k : CW * (k + 1)],
                            start=False, stop=(k == KT - 1),
                        )
                        nc.tensor.matmul(
                            ps_sq0, ones_b, sq0[k], start=False, stop=(k == KT - 1)
                        )
                pm0, psq0 = ps_mu0, ps_sq0

                # thr path: free-axis reduces of the broadcast psum
                for c in range(4):
                    nc.vector.tensor_reduce(
                        wred[:, c : c + 1], ps_w[c], axis=Ax.X, op=Alu.add
                    )
                for c in range(4):
                    nc.vector.tensor_reduce(
                        wred[:, 4 + c : 5 + c], ps_w[c], axis=Ax.X, op=Alu.max
                    )
                tot_b = scal[:, 2:3]
                nc.vector.tensor_reduce(tot_b, wred[:, 0:4], axis=Ax.X, op=Alu.add)
                beta_b = scal[:, 3:4]
                nc.vector.tensor_reduce(beta_b, wred[:, 4:8], axis=Ax.X, op=Alu.max)
                # two_thr = gamma_w/SCALE_W = tot/(D*O*SCALE_W) + EPS/SCALE_W
                two_thr = scal[:, 4:5]
                nc.scalar.activation(
                    two_thr, tot_b, Act.Copy,
                    bias=float(EPS / SCALE_W), scale=float(1.0 / (D * O * SCALE_W)),
                )
                inv2t = scal[:, 5:6]
                nc.vector.reciprocal(inv2t, two_thr)

                # chunk-0 mu / subtracts (gpsimd chases ahead of the GEMM)
                mu0, var0 = mu_var_part(0, pm0, psq0)
                subs_part(0, mu0)

                # ---- ternarize: q16 = round(v*inv2t); bw = clip -> fp8
                bwt = [None] * KT

                def tern(k):
                    wk = w16t[k // 2][:, O * (k % 2) : O * (k % 2 + 1)]
                    q16 = q16p.tile([P, O], I16, tag="q16")
                    nc.vector.tensor_scalar(
                        out=q16, in0=wk, scalar1=inv2t, scalar2=None, op0=Alu.mult
                    )
                    bwk = bwp.tile([P, O], FP8, tag="bw", name=f"bw{k}")
                    nc.vector.tensor_scalar(
                        out=bwk, in0=q16, scalar1=-1.0, scalar2=1.0,
                        op0=Alu.max, op1=Alu.min,
                    )
                    bwt[k] = bwk

                for k in KS_ORDER[:4]:
                    tern(k)
                rstd_part(0, var0)  # var already computed; sd/rstd/columnize
                for k in KS_ORDER[4:]:
                    tern(k)
                rbb_part(0)  # executes once beta + columnize land

            # ---- w16 released: its SBUF hosts chunks 2,3 -------------
            with tc.tile_pool(name="xb23", bufs=2) as xb23:
                xb[2] = xb23.tile([P, KT * CW], BF16, name="xc2")
                nc.sync.dma_start(xb[2], xc[2 * P : 3 * P, :])
                xb[3] = xb23.tile([P, KT * CW], BF16, name="xc3")
                nc.sync.dma_start(xb[3], xc[3 * P : 4 * P, :])

                def epilogue(m, j, pys):
                    g = 4 * m + j
                    for c in range(NCH):
                        ysb = ypool.tile([P, CH], F32, tag="y")
                        nc.scalar.mul(ysb, pys[c], rbb[:, g : g + 1])
                        nc.sync.dma_start(
                            y[P * g : P * (g + 1), CH * c : CH * (c + 1)], ysb
                        )

                def gemm_pair(m, j0, j1):
                    """Two m-tiles k-interleaved across all 8 PSUM banks,
                    pacing the PE to the ternarize trickle."""
                    js = [j0, j1]
                    pys = {}
                    for j in js:
                        g = 4 * m + j
                        pys[j] = [
                            psum.tile([P, CH], F32, tag="ps", name=f"py{g}_{c}")
                            for c in range(NCH)
                        ]
                    for i, k in enumerate(KS_ORDER):
                        first, last = i == 0, i == KT - 1
                        for j in js:
                            lhs = xb[m][:, CW * k + P * j : CW * k + P * (j + 1)]
                            for c in range(NCH):
                                nc.tensor.matmul(
                                    pys[j][c], lhs,
                                    bwt[k][:, CH * c : CH * (c + 1)],
                                    start=first, stop=last,
                                )
                    for j in js:
                        epilogue(m, j, pys[j])

                def gemm_one(m, j, pre=None, last=False):
                    if pre:
                        pre()
                    g = 4 * m + j
                    pys = [
                        psum.tile([P, CH], F32, tag="ps", name=f"py{g}_{c}")
                        for c in range(NCH)
                    ]
                    if not last:
                        for i, k in enumerate(KS_ORDER):
                            lhs = xb[m][:, CW * k + P * j : CW * k + P * (j + 1)]
                            first, last_k = i == 0, i == KT - 1
                            for c in range(NCH):
                                nc.tensor.matmul(
                                    pys[c], lhs, bwt[k][:, CH * c : CH * (c + 1)],
                                    start=first, stop=last_k,
                                )
                        epilogue(m, j, pys)
                        return
                    # final m-tile: c-major so banks finish (and drain)
                    # progressively; split muls/DMAs across engines/queues
                    engs = (
                        (nc.vector, nc.sync), (nc.scalar, nc.scalar),
                        (nc.vector, nc.sync), (nc.scalar, nc.scalar),
                    )
                    for c in range(NCH):
                        for i, k in enumerate(KS_ORDER):
                            lhs = xb[m][:, CW * k + P * j : CW * k + P * (j + 1)]
                            nc.tensor.matmul(
                                pys[c], lhs, bwt[k][:, CH * c : CH * (c + 1)],
                                start=(i == 0), stop=(i == KT - 1),
                            )
                        ysb = ypool.tile([P, CH], F32, tag="y", name=f"yfin{c}")
                        meng, deng = engs[c]
                        if meng is nc.vector:
                            nc.vector.tensor_scalar(
                                out=ysb, in0=pys[c], scalar1=rbb[:, g : g + 1],
                                scalar2=None, op0=Alu.mult,
                            )
                        else:
                            nc.scalar.mul(ysb, pys[c], rbb[:, g : g + 1])
                        deng.dma_start(
                            y[P * g : P * (g + 1), CH * c : CH * (c + 1)], ysb
                        )

                carry = {}

                def stats_ins(m, sq_eng):
                    def _f():
                        sqs = sq_tiles(m, range(KT), sq_eng)
                        pm, psq = stats_mms(m, sqs)
                        mu_m, var_m = mu_var_part(m, pm, psq)
                        subs_part(m, mu_m)
                        carry[m] = var_m
                    return _f

                def fin_ins(m):
                    def _f():
                        rstd_part(m, carry.pop(m))
                        rbb_part(m)
                    return _f

                gemm_pair(0, 0, 1)
                gemm_one(0, 2, pre=stats_ins(1, "s"))
                gemm_one(0, 3, pre=fin_ins(1))
                gemm_one(1, 0, pre=stats_ins(2, "v"))
                gemm_one(1, 1, pre=fin_ins(2))
                gemm_one(1, 2, pre=stats_ins(3, "v"))
                gemm_one(1, 3, pre=fin_ins(3))
                for j in range(4):
                    gemm_one(2, j)
                for j in range(4):
                    gemm_one(3, j, last=(j == 3))

    nc.compile()
    return nc


_NC_CACHE = None


def _get_nc():
    global _NC_CACHE
    if _NC_CACHE is None:
        _NC_CACHE = build_nc()
    return _NC_CACHE


def _sr_fp8_abs(a, seed=12345):
    """Stochastic-round |a| to fp8e4m3 (unbiased, unlike RNE)."""
    import ml_dtypes

    aw = np.abs(a).astype(np.float32)
    f8 = aw.astype(ml_dtypes.float8_e4m3fn)
    lo_c = f8.astype(np.float32)
    hi = np.nextafter(f8, ml_dtypes.float8_e4m3fn(448)).astype(np.float32)
    lo_dn = np.nextafter(f8, ml_dtypes.float8_e4m3fn(0)).astype(np.float32)
    lo3 = np.where(lo_c <= aw, lo_c, lo_dn)
    hi2 = np.where(lo_c >= aw, lo_c, hi)
    span = np.where(hi2 > lo3, hi2 - lo3, 1.0)
    p = (aw - lo3) / span
    rng = np.random.default_rng(seed)
    out = np.where(rng.random(aw.shape) < p, hi2, lo3)
    return out.astype(ml_dtypes.float8_e4m3fn)


def _prep_in_maps(x, fweight):
    import ml_dtypes

    x2 = np.asarray(x, dtype=np.float32).reshape(N_TOK, D).astype(ml_dtypes.bfloat16)
    fw = np.asarray(fweight, dtype=np.float32)
    fwt = fw.T  # [D, O]
    wi = np.round(fwt / SCALE_W).clip(-32767, 32767).astype(np.int16)
    # contiguous pieces: piece q rows = k-tiles 2q,2q+1 -> [8*128, 4096]
    wi = np.ascontiguousarray(
        wi.reshape(NWQ, 2, P, O).transpose(0, 2, 1, 3).reshape(NWQ * P, 2 * O)
    )
    w8 = np.ascontiguousarray(_sr_fp8_abs(fw))  # [O, D] = [16*128, 2048]
    in_maps = []
    for c in range(N_CORES):
        xs = x2[c * TOK : (c + 1) * TOK, :]  # [TOK, D] bf16
        chunks = []
        for m in range(NC_CHUNK):
            blk = xs[m * CW : (m + 1) * CW, :].T  # [D, CW]
            chunks.append(
                blk.reshape(KT, P, CW).transpose(1, 0, 2).reshape(P, KT * CW)
            )
        xch = np.ascontiguousarray(np.concatenate(chunks, axis=0))
        in_maps.append({"xc": xch, "wi": wi, "w8": w8})
    return in_maps


def run_spmd(x, fweight, **kw):
    nc = _get_nc()
    in_maps = _prep_in_maps(x, fweight)
    return run_bass_kernel_spmd(nc, in_maps, core_ids=list(range(N_CORES)), **kw)


def kernel(x, fweight):
    res = run_spmd(x, fweight)
    y = np.concatenate([res.results[c]["y"] for c in range(N_CORES)], axis=0)
    return y.reshape(4, 4096, O)


if __name__ == "__main__":
    xx = np.random.randn(4, 4096, D).astype(np.float32)
    ww = np.random.uniform(-1 / np.sqrt(D), 1 / np.sqrt(D), (O, D)).astype(np.float32)
    out = kernel(xx, ww)
    print("out", out.shape, out.dtype, float(np.abs(out).mean()))
